# revision 16
# baseline (speedup 1.0000x reference)
"""Trainium2 Bass kernel for nn_CNNTransformer (CNN-stem + 3 conv-attention
transformer blocks). Pure data-parallel over batch: 8 images -> 8 NeuronCores,
weights replicated, no collectives.

Self-contained: hardcodes all shapes; host-side numpy does weight repacking
(fold layernorm scales into conv weights, fold 1/sqrt(hd) into q weights,
fold 1/16 pool scale into the linear, bias rows/columns for matmul bias
tricks), then builds/compiles one Bass program and runs it SPMD on cores 0-7
via run_bass_kernel_spmd.

Layout: activations as [channel partitions, spatial free]; convs are
shift-accumulated matmuls reading zero-padded SBUF tiles through strided APs.
Attention computes transposed scores s^T[m,n] = k^T q so the softmax sum is
a matmul (ones column appended to transposed V -> row 32 of the y-psum is
the normalizer Z). Residual stream stays f32; matmuls are bf16 except the
large-logit heads (2..5) which run in float32r.
"""
import sys

sys.path.insert(0, "/opt/trn_rl_repo")

import numpy as np
import ml_dtypes
from contextlib import ExitStack

BF = ml_dtypes.bfloat16


def to_f32r(a):
    """Round f32 -> fp32r (TF32-like: 8-bit exp, 11-bit mantissa; low 12 bits 0)."""
    u = np.ascontiguousarray(a, dtype=np.float32).view(np.uint32)
    u2 = (u + np.uint32(0x7FF) + ((u >> np.uint32(12)) & np.uint32(1))) & np.uint32(0xFFFFF000)
    return u2.view(np.float32)

HEAD_N = 8
HD = 32
KS = [1, 3, 5, 7, 9, 11, 1, 3]          # per-head conv kernel size
PADS = [0, 1, 2, 3, 4, 5, 0, 1]
PREC_HEADS = frozenset({2, 3, 4, 5})     # heads computed in f32r (large logits)
NBLOCKS = 3
QPAD = 5          # halo for attention conv input (max pad)
MT = 7            # m tiles of 112


def _prep(x, params):
    """Host-side numpy weight repack. Returns (per_core_x, shared_tensors)."""
    f32 = np.float32
    g = lambda a: np.asarray(a, dtype=f32)
    x = g(x)
    t = {}
    t["pos"] = to_f32r(g(params["pos"])[0, 0])              # [224,224]

    # conv1 lhsT per kx: rows p = ci*3+ky (ci<3) | 9+ky (pos); cols kx*256+co
    ew1 = g(params["ew1"])                                  # [256,3,3,3]
    w1b = np.zeros((12, 3 * 256), f32)
    for ky in range(3):
        for kx in range(3):
            for ci in range(3):
                w1b[ci * 3 + ky, kx * 256:(kx + 1) * 256] = ew1[:, ci, ky, kx]
            w1b[9 + ky, kx * 256:(kx + 1) * 256] = ew1[:, :, ky, kx].sum(1)
    t["w1b"] = to_f32r(w1b)

    # conv2: [128, 9*512]; slice (s, cit, ct)
    t["w2"] = to_f32r(g(params["ew2"]).transpose(1, 2, 3, 0).reshape(2, 128, 9, 256)
                      .transpose(1, 2, 0, 3).reshape(128, 9 * 512))

    lw = g(params["lw"]) / 16.0                             # [784,196] (pool scale folded)
    lb = g(params["lb"])                                    # [784]
    t["lw0"] = to_f32r(lw.T[0:98])
    t["lw1"] = to_f32r(lw.T[98:196])
    t["lbr"] = lb.copy()

    # per-partition biases packed as one [128, 40] matrix
    bias_mat = np.zeros((128, 40), f32)
    bias_mat[:, 0:2] = g(params["eb1"]).reshape(2, 128).T
    bias_mat[:, 2:4] = g(params["eb2"]).reshape(2, 128).T

    abt = np.zeros((NBLOCKS, 8, 96), f32)
    for b, blk in enumerate(params["blocks"]):
        ln1 = g(blk["ln1"])
        ln2 = g(blk["ln2"])
        for h in range(HEAD_N):
            k = KS[h]
            aw = g(blk["aw"][h]) * ln1[None, :, None, None]   # [96,256,k,k]
            aw = aw.copy()
            aw[0:32] *= 1.0 / np.sqrt(np.float32(HD))
            arr = (aw.transpose(1, 2, 3, 0).reshape(2, 128, k * k, 96)
                   .transpose(1, 2, 0, 3).reshape(128, k * k * 192))
            t[f"aw{b}_{h}"] = to_f32r(arr)
            ab = g(blk["ab"][h]).copy()
            ab[0:32] *= 1.0 / np.sqrt(np.float32(HD))
            abt[b, h] = ab
        t[f"pw{b}"] = to_f32r(g(blk["pw"])[:, :, 0, 0].T.reshape(2, 128, 256)
                              .transpose(1, 0, 2).reshape(128, 512))
        bias_mat[:, 4 + b * 2:4 + b * 2 + 2] = g(blk["pb"]).reshape(2, 128).T
        mw1 = g(blk["mw1"]) * ln2[None, :, None, None]        # [1024,256,3,3]
        t[f"mw1{b}"] = to_f32r(mw1.transpose(1, 2, 3, 0).reshape(2, 128, 9, 1024)
                               .transpose(1, 2, 0, 3).reshape(128, 9 * 2048))
        bias_mat[:, 10 + b * 8:10 + b * 8 + 8] = g(blk["mb1"]).reshape(8, 128).T
        t[f"mw2{b}"] = to_f32r(g(blk["mw2"]).transpose(1, 2, 3, 0).reshape(8, 128, 9, 256)
                               .transpose(1, 2, 0, 3).reshape(128, 9 * 2048))
        bias_mat[:, 34 + b * 2:34 + b * 2 + 2] = g(blk["mb2"]).reshape(2, 128).T
    t["abt"] = to_f32r(abt.reshape(NBLOCKS * 8 * 96))
    t["bias_mat"] = bias_mat
    t["lnfw"] = g(params["lnf_w"]).reshape(784).copy()
    t["lnfb"] = g(params["lnf_b"]).reshape(784).copy()
    xs = [to_f32r(x[i]) for i in range(x.shape[0])]
    return xs, t


def build(debug_taps=False):
    from concourse import bass, bacc, mybir, tile
    from concourse.masks import make_identity

    F32 = mybir.dt.float32
    F32R = mybir.dt.float32r
    BF16 = mybir.dt.bfloat16
    AF = mybir.ActivationFunctionType
    OP = mybir.AluOpType

    nc = bacc.Bacc("TRN2", target_bir_lowering=False, debug=False,
                   enable_asserts=True)

    def din(name, shape, dt=F32):
        return nc.dram_tensor(name, list(shape), dt, kind="ExternalInput").ap()

    x_d = din("x", [3, 224, 224], F32R)
    pos_d = din("pos", [224, 224], F32R)
    w1b_d = din("w1b", [12, 768], F32R)
    w2_d = din("w2", [128, 9 * 512], F32R)
    lw0_d = din("lw0", [98, 784], F32R)
    lbr_d = din("lbr", [784])
    lw1_d = din("lw1", [98, 784], F32R)
    aw_d = {}
    for b in range(NBLOCKS):
        for h in range(HEAD_N):
            k = KS[h]
            aw_d[(b, h)] = din(f"aw{b}_{h}", [128, k * k * 192], F32R)
    pw_d = [din(f"pw{b}", [128, 512], F32R) for b in range(NBLOCKS)]
    mw1_d = [din(f"mw1{b}", [128, 9 * 2048], F32R) for b in range(NBLOCKS)]
    mw2_d = [din(f"mw2{b}", [128, 9 * 2048], F32R) for b in range(NBLOCKS)]
    abt_d = din("abt", [NBLOCKS * 8 * 96], F32R)
    bias_d = din("bias_mat", [128, 40])
    lnfw_d = din("lnfw", [784])
    lnfb_d = din("lnfb", [784])
    out_d = nc.dram_tensor("out", [256, 784], F32, kind="ExternalOutput").ap()
    tap_d = {}
    if debug_taps:
        for nm in ["res_stem", "res_a0", "res_b0", "res_b1", "res_b2"]:
            tap_d[nm] = nc.dram_tensor(nm, [256, 784], F32R, kind="ExternalOutput").ap()

    def dap(dram_ap, offset, dims):
        return bass.AP(tensor=dram_ap.tensor, offset=offset, ap=[list(d) for d in dims])

    with tile.TileContext(nc) as tc, ExitStack() as ctx:
        const = ctx.enter_context(tc.tile_pool(name="const", bufs=1))
        pers = ctx.enter_context(tc.tile_pool(name="pers", bufs=1))

        # ---------- constants ----------
        ident = const.tile([128, 128], BF16, tag="ident")
        make_identity(nc, ident)
        ident32 = const.tile([128, 128], F32, tag="ident32")
        make_identity(nc, ident32)
        identR = const.tile([128, 128], F32R, tag="identR")
        nc.vector.tensor_copy(identR, ident32)
        ones128 = const.tile([128, 128], F32R, tag="ones128")
        nc.vector.memset(ones128.bitcast(F32), 1.0)
        epst = const.tile([128, 1], F32, tag="epst")
        nc.vector.memset(epst, 1e-5)
        bias_t = const.tile([128, 40], F32, tag="bias_t")
        nc.sync.dma_start(bias_t, bias_d)
        ab_row = const.tile([1, NBLOCKS * 8 * 96], F32R, tag="ab_row")
        nc.sync.dma_start(ab_row, dap(abt_d, 0, [[0, 1], [1, NBLOCKS * 8 * 96]]))
        ones_r32 = const.tile([1, 392], F32R, tag="ones_r32")
        nc.vector.memset(ones_r32.bitcast(F32), 1.0)
        ones_r16 = const.tile([1, 392], BF16, tag="ones_r16")
        nc.vector.memset(ones_r16, 1.0)
        lnfw_r = const.tile([1, 784], F32, tag="lnfw_r")
        nc.sync.dma_start(lnfw_r, dap(lnfw_d, 0, [[0, 1], [1, 784]]))
        lnfb_r = const.tile([1, 784], F32, tag="lnfb_r")
        nc.sync.dma_start(lnfb_r, dap(lnfb_d, 0, [[0, 1], [1, 784]]))
        lnfw_b = const.tile([128, 784], F32, tag="lnfw_b")
        nc.gpsimd.partition_broadcast(lnfw_b, lnfw_r)
        lnfb_b = const.tile([128, 784], F32, tag="lnfb_b")
        nc.gpsimd.partition_broadcast(lnfb_b, lnfb_r)
        lbr_r = const.tile([1, 784], F32, tag="lbr_r")
        nc.sync.dma_start(lbr_r, dap(lbr_d, 0, [[0, 1], [1, 784]]))
        lbF = const.tile([1, 784], F32R, tag="lbF")
        nc.vector.tensor_copy(lbF, lbr_r)

        def bias_ap(col):
            return bias_t[:, col:col + 1]

        # ---------- persistent activation tiles ----------
        res = [pers.tile([128, 784], F32R, tag=f"res{t}", name=f"res{t}") for t in range(2)]

        def msz(ap):
            if ap.dtype == F32R:
                ap = ap.bitcast(F32)
            nc.vector.memset(ap, 0.0)

        def zero_border(tl, n, halo):
            msz(tl[:, 0:halo, :])
            msz(tl[:, n - halo:n, :])
            msz(tl[:, halo:n - halo, 0:halo])
            msz(tl[:, halo:n - halo, n - halo:n])

        # ================= STEM =================
        with tc.tile_pool(name="stem", bufs=1) as stem, \
             tc.tile_pool(name="stemps", bufs=8, space="PSUM") as stemps:
            h1 = [stem.tile([128, 114, 114], F32R, tag=f"h1_{t}", name=f"h1_{t}") for t in range(2)]
            for t in range(2):
                zero_border(h1[t], 114, 1)
            w1s = stem.tile([12, 768], F32R, tag="w1s")
            nc.sync.dma_start(w1s, w1b_d)

            # conv1 (+pos as 4th input "channel"); K=(ci,ky)=12, kx via 3 shifts.
            # x_rep2[p=(ci*3+ky | 9+ky), oy, 1+c] = x[ci, 2*oy+ky-1, c] (stride-2
            # rows pre-gathered by DMA; stride-2 cols gathered by the matmul AP)
            for strip in range(7):               # 16 output rows each
                oyb = strip * 16
                xr = stem.tile([12, 16, 226], F32R, tag="xr", name="xr", bufs=1)
                msz(xr[:, :, 0:1])
                if strip == 0:
                    msz(xr[:, 0:1, :])   # ky!=0 partitions re-DMA'd below
                for ci in range(4):
                    src = x_d if ci < 3 else pos_d
                    base = ci * 50176 if ci < 3 else 0
                    for ky in range(3):
                        p = ci * 3 + ky
                        oy0 = 1 if (ky == 0 and strip == 0) else 0
                        off = base + (2 * (oyb + oy0) + ky - 1) * 224
                        nc.sync.dma_start(
                            xr[p:p + 1, oy0:16, 1:225],
                            dap(src, off, [[0, 1], [448, 16 - oy0], [1, 224]]))
                for ct in range(2):
                    for nt in range(4):          # 4 output rows per matmul
                        ps = stemps.tile([128, 448], F32, tag="ps", name="ps")
                        for kx in range(3):
                            rhs = bass.AP(
                                tensor=xr.tensor,
                                offset=xr.offset + (nt * 4) * 226 + kx,
                                ap=[[16 * 226, 12], [226, 4], [2, 112]])
                            nc.tensor.matmul(
                                ps, w1s[:, kx * 256 + ct * 128:kx * 256 + (ct + 1) * 128],
                                rhs, start=(kx == 0), stop=(kx == 2))
                        r0 = 1 + oyb + nt * 4
                        nc.scalar.activation(h1[ct][:, r0:r0 + 4, 1:113], ps,
                                             AF.Gelu, bias=bias_ap(ct), scale=1.0)

            # conv2: 256->256 3x3 stride2 -> 56^2, two 4-row-group streams
            h2 = [stem.tile([128, 3136], F32R, tag=f"h2_{t}", name=f"h2_{t}") for t in range(2)]
            for sg in range(2):
                pss = {}
                for ct in range(2):
                    for nt in range(4):
                        pss[(ct, nt)] = stemps.tile([128, 392], F32, tag="ps", name="ps")
                for s in range(9):
                    ky, kx = s // 3, s % 3
                    wt = stem.tile([128, 512], F32R, tag="w2t", name="w2t", bufs=3)
                    nc.sync.dma_start(wt, dap(w2_d, s * 512, [[9 * 512, 128], [1, 512]]))
                    for cit in range(2):
                        for ct in range(2):
                            for nt in range(4):
                                gnt = sg * 4 + nt
                                rhs = bass.AP(
                                    tensor=h1[cit].tensor,
                                    offset=h1[cit].offset + (14 * gnt + ky) * 114 + kx,
                                    ap=[[114 * 114, 128], [228, 7], [2, 56]])
                                nc.tensor.matmul(
                                    pss[(ct, nt)],
                                    wt[:, cit * 256 + ct * 128:cit * 256 + (ct + 1) * 128],
                                    rhs,
                                    start=(s == 0 and cit == 0), stop=(s == 8 and cit == 1))
                for ct in range(2):
                    for nt in range(4):
                        gnt = sg * 4 + nt
                        nc.scalar.activation(h2[ct][:, gnt * 392:(gnt + 1) * 392],
                                             pss[(ct, nt)], AF.Gelu,
                                             bias=bias_ap(2 + ct), scale=1.0)

            # 4x4 pool (sum; /16 folded into lw) -> [128,196] bf16
            pooled = []
            for t in range(2):
                hr = h2[t].rearrange("p (h pw dx) -> p h pw dx", h=56, dx=4)
                st1 = stem.tile([128, 56, 14], F32, tag="st1", name="st1", bufs=2)
                nc.vector.tensor_add(st1, hr[:, :, :, 0], hr[:, :, :, 1])
                nc.vector.tensor_add(st1, st1, hr[:, :, :, 2])
                nc.vector.tensor_add(st1, st1, hr[:, :, :, 3])
                st2 = stem.tile([128, 14, 14], F32, tag="st2", name="st2", bufs=2)
                nc.vector.tensor_add(st2, st1[:, 0::4, :], st1[:, 1::4, :])
                nc.vector.tensor_add(st2, st2, st1[:, 2::4, :])
                pl = stem.tile([128, 196], F32R, tag=f"pooled{t}", name=f"pooled{t}")
                nc.vector.tensor_add(pl.rearrange("p (a b) -> p a b", a=14),
                                     st2, st1[:, 3::4, :])
                pooled.append(pl)

            # transpose pooled -> linear lhsT tiles (ones row in kt=0 adds lb)
            ht = [[None, None], [None, None]]
            for ct in range(2):
                for kt in range(2):
                    pst = stemps.tile([98, 128], F32R, tag="ps", name="ps")
                    nc.tensor.transpose(pst, pooled[ct][:, kt * 98:(kt + 1) * 98], identR)
                    hh = stem.tile([98, 128], F32R, tag=f"ht{kt}_{ct}", name=f"ht{kt}_{ct}")
                    nc.scalar.copy(hh, pst)
                    ht[kt][ct] = hh

            lw0t = stem.tile([98, 784], F32R, tag="lw0t")
            nc.sync.dma_start(lw0t, lw0_d)
            lw1t = stem.tile([98, 784], F32R, tag="lw1t")
            nc.sync.dma_start(lw1t, lw1_d)
            for ct in range(2):
                for nh in range(2):
                    ps = stemps.tile([128, 392], F32, tag="ps", name="ps")
                    sl = slice(nh * 392, (nh + 1) * 392)
                    nc.tensor.matmul(ps, ones_r32[:, 0:128], lbF[:, sl],
                                     start=True, stop=False)
                    nc.tensor.matmul(ps, ht[0][ct], lw0t[:, sl], start=False, stop=False)
                    nc.tensor.matmul(ps, ht[1][ct], lw1t[:, sl], start=False, stop=True)
                    nc.scalar.copy(res[ct][:, sl], ps)

        def dump_tap(nm):
            if debug_taps:
                for t_ in range(2):
                    nc.sync.dma_start(tap_d[nm][t_ * 128:(t_ + 1) * 128, :], res[t_])

        dump_tap("res_stem")

        # ================= BLOCKS =================
        with tc.tile_pool(name="bpers", bufs=1) as bpers, \
             tc.tile_pool(name="wpool", bufs=2) as wpool, \
             tc.tile_pool(name="sb", bufs=2) as sb, \
             tc.tile_pool(name="epool", bufs=7) as epool, \
             tc.tile_pool(name="vpool", bufs=7) as vpool, \
             tc.tile_pool(name="psB", bufs=8, space="PSUM") as psB:

            xq32 = [bpers.tile([128, 38, 38], F32R, tag=f"xq32_{t}", name=f"xq32_{t}") for t in range(2)]
            xm = [bpers.tile([128, 30, 30], F32R, tag=f"xm{t}", name=f"xm{t}") for t in range(2)]
            hm = [bpers.tile([128, 30, 30], F32R, tag=f"hm{t}", name=f"hm{t}") for t in range(8)]
            yall = [bpers.tile([128, 784], F32R, tag=f"yall{t}", name=f"yall{t}") for t in range(2)]
            for t in range(2):
                zero_border(xq32[t], 38, QPAD)
                zero_border(xm[t], 30, 1)
            for t in range(8):
                zero_border(hm[t], 30, 1)

            def fnorm(b, dst, halo, npad):
                """FeatureNorm over channels -> interiors of padded tiles."""
                x2 = [sb.tile([128, 784], F32R, tag="x2", name="x2") for _ in range(2)]
                for t in range(2):
                    nc.vector.tensor_mul(x2[t], res[t], res[t])
                m_b = sb.tile([128, 784], F32, tag="m_b", bufs=1)
                var_b = sb.tile([128, 784], F32, tag="var_b", bufs=1)
                for nh in range(2):
                    sl = slice(nh * 392, (nh + 1) * 392)
                    s_ps = psB.tile([128, 392], F32, tag="ps", name="ps")
                    nc.tensor.matmul(s_ps, ones128, res[0][:, sl],
                                     start=True, stop=False)
                    nc.tensor.matmul(s_ps, ones128, res[1][:, sl],
                                     start=False, stop=True)
                    ss_ps = psB.tile([128, 392], F32, tag="ps", name="ps")
                    nc.tensor.matmul(ss_ps, ones128, x2[0][:, sl],
                                     start=True, stop=False)
                    nc.tensor.matmul(ss_ps, ones128, x2[1][:, sl],
                                     start=False, stop=True)
                    nc.scalar.activation(m_b[:, sl], s_ps, AF.Copy, bias=0.0, scale=1.0 / 256.0)
                    msq = sb.tile([128, 392], F32, tag="msq", name="msq")
                    nc.vector.tensor_mul(msq, m_b[:, sl], m_b[:, sl])
                    nc.vector.scalar_tensor_tensor(var_b[:, sl], ss_ps, 1.0 / 256.0, msq,
                                                   op0=OP.mult, op1=OP.subtract)
                sd_b = sb.tile([128, 784], F32, tag="sd_b", bufs=1)
                nc.scalar.activation(sd_b, var_b, AF.Sqrt, bias=epst, scale=1.0)
                a_b = sb.tile([128, 784], F32, tag="a_b", bufs=1)
                nc.vector.reciprocal(a_b, sd_b)
                lo, hi = halo, npad - halo
                for t in range(2):
                    tmp = sb.tile([128, 784], F32, tag="tmp", name="tmp")
                    nc.vector.tensor_sub(tmp, res[t], m_b)
                    tr = tmp.rearrange("p (a b) -> p a b", a=28)
                    ar = a_b.rearrange("p (a b) -> p a b", a=28)
                    nc.vector.tensor_mul(dst[t][:, lo:hi, lo:hi], tr, ar)

            def conv_shift_ap(tl, npad, ky, kx, pad, nh):
                halo = QPAD if npad == 38 else 1
                rr = ky - pad + halo + 14 * nh
                cc = kx - pad + halo
                return bass.AP(tensor=tl.tensor,
                               offset=tl.offset + rr * npad + cc,
                               ap=[[npad * npad, 128], [npad, 14], [1, 28]])

            for b in range(NBLOCKS):
                # ---- attention ----
                fnorm(b, xq32, QPAD, 38)

                vta = [vpool.tile([112, 8 * 33], F32R, tag="vta", name="vta") for _ in range(MT)]
                for mt in range(MT):
                    nc.vector.memset(vta[mt][:, 32::33].bitcast(F32), 1.0)

                for h in range(HEAD_N):
                    k, pad = KS[h], PADS[h]
                    nsh = k * k
                    act = xq32
                    ab_sl = ab_row[:, (b * 8 + h) * 96:(b * 8 + h + 1) * 96]
                    ones_r = ones_r32

                    qkv = sb.tile([96, 784], F32R, tag="qkv32", name="qkv")
                    ps_pair = [psB.tile([96, 392], F32, tag="ps", name="ps")
                               for _ in range(2)]
                    for nh in range(2):
                        nc.tensor.matmul(ps_pair[nh], ab_sl, ones_r,
                                         start=True, stop=False)
                    CH = 4
                    for s0 in range(0, nsh, CH):
                        ch = min(CH, nsh - s0)
                        wt = wpool.tile([128, CH * 192], F32R, tag="awt32", name="awt")
                        nc.sync.dma_start(
                            wt[:, 0:ch * 192],
                            dap(aw_d[(b, h)], s0 * 192,
                                [[nsh * 192, 128], [1, ch * 192]]))
                        for si in range(ch):
                            s = s0 + si
                            ky, kx = s // k, s % k
                            for cit in range(2):
                                lhs = wt[:, si * 192 + cit * 96:si * 192 + (cit + 1) * 96]
                                for nh in range(2):
                                    rhs = conv_shift_ap(act[cit], 38, ky, kx, pad, nh)
                                    nc.tensor.matmul(
                                        ps_pair[nh], lhs, rhs, start=False,
                                        stop=(s == nsh - 1 and cit == 1))
                    for nh in range(2):
                        nc.scalar.copy(qkv[:, nh * 392:(nh + 1) * 392], ps_pair[nh])
                    # k must sit at base partition 0 for the scores matmul
                    kt = sb.tile([32, 784], F32R, tag="kt32", name="kt")
                    nc.sync.dma_start(kt, qkv[32:64, :])

                    # transpose v (rows 64:96; identity sliced at matching base)
                    for mt in range(MT):
                        pst = psB.tile([112, 32], F32R, tag="ps", name="ps")
                        nc.tensor.transpose(pst, qkv[64:96, mt * 112:(mt + 1) * 112],
                                            identR[64:96, 64:96])
                        nc.scalar.copy(vta[mt][:, h * 33:h * 33 + 32], pst)

                    # transposed scores + exp
                    eh = [epool.tile([112, 784], F32R, tag="eh", name="eh")
                          for _ in range(MT)]
                    for mt in range(MT):
                        for nh in range(2):
                            sl = slice(nh * 392, (nh + 1) * 392)
                            ps = psB.tile([112, 392], F32, tag="ps", name="ps")
                            nc.tensor.matmul(ps, kt[:, mt * 112:(mt + 1) * 112],
                                             qkv[0:32, sl], start=True, stop=True)
                            nc.scalar.activation(eh[mt][:, sl], ps, AF.Exp)

                    # y (rows 0:32) + Z (row 32); normalize; DMA into yall rows
                    for nh in range(2):
                        sl = slice(nh * 392, (nh + 1) * 392)
                        ps = psB.tile([33, 392], F32, tag="ps", name="ps")
                        for mt in range(MT):
                            nc.tensor.matmul(ps, vta[mt][:, h * 33:h * 33 + 33],
                                             eh[mt][:, sl],
                                             start=(mt == 0), stop=(mt == MT - 1))
                        zr = sb.tile([1, 392], F32, tag="zr", name="zr", bufs=1)
                        nc.vector.reciprocal(zr, ps[32:33, :])
                        zb = sb.tile([32, 392], F32, tag="zb", name="zb", bufs=1)
                        nc.gpsimd.partition_broadcast(zb, zr)
                        yn = sb.tile([32, 392], F32R, tag="yn", name="yn")
                        nc.vector.tensor_mul(yn, ps[0:32, :], zb)
                        nc.sync.dma_start(
                            yall[h // 4][(h % 4) * 32:(h % 4) * 32 + 32, sl], yn)

                # c_proj + residual
                pwt = wpool.tile([128, 512], F32R, tag="pwt", name="pwt")
                nc.sync.dma_start(pwt, pw_d[b])
                for ct in range(2):
                    for nh in range(2):
                        sl = slice(nh * 392, (nh + 1) * 392)
                        ps = psB.tile([128, 392], F32, tag="ps", name="ps")
                        for cit in range(2):
                            nc.tensor.matmul(
                                ps, pwt[:, cit * 256 + ct * 128:cit * 256 + (ct + 1) * 128],
                                yall[cit][:, sl],
                                start=(cit == 0), stop=(cit == 1))
                        nc.vector.scalar_tensor_tensor(
                            res[ct][:, sl], ps, bias_ap(4 + b * 2 + ct),
                            res[ct][:, sl], op0=OP.add, op1=OP.add)

                if b == 0:
                    dump_tap("res_a0")

                # ---- MLP ----
                fnorm(b, xm, 1, 30)
                for cog in range(2):             # 4 co-tiles per group
                    pss = {}
                    for co in range(4):
                        for nh in range(2):
                            pss[(co, nh)] = psB.tile([128, 392], F32, tag="ps", name="ps")
                    for s in range(9):
                        ky, kx = s // 3, s % 3
                        wt = wpool.tile([128, 2048], F32R, tag="mw1t", name="mw1t",
                                        bufs=2)
                        nc.sync.dma_start(wt, dap(mw1_d[b], s * 2048,
                                                  [[9 * 2048, 128], [1, 2048]]))
                        for cit in range(2):
                            for co in range(4):
                                cot = cog * 4 + co
                                for nh in range(2):
                                    nc.tensor.matmul(
                                        pss[(co, nh)],
                                        wt[:, cit * 1024 + cot * 128:cit * 1024 + (cot + 1) * 128],
                                        conv_shift_ap(xm[cit], 30, ky, kx, 1, nh),
                                        start=(s == 0 and cit == 0),
                                        stop=(s == 8 and cit == 1))
                    for co in range(4):
                        cot = cog * 4 + co
                        for nh in range(2):
                            out_ap = bass.AP(
                                tensor=hm[cot].tensor,
                                offset=hm[cot].offset + (1 + 14 * nh) * 30 + 1,
                                ap=[[900, 128], [30, 14], [1, 28]])
                            nc.scalar.activation(out_ap, pss[(co, nh)], AF.Gelu,
                                                 bias=bias_ap(10 + b * 8 + cot),
                                                 scale=1.0)
                pss = {}
                for ct in range(2):
                    for nh in range(2):
                        pss[(ct, nh)] = psB.tile([128, 392], F32, tag="ps", name="ps")
                for s in range(9):
                    ky, kx = s // 3, s % 3
                    wt = wpool.tile([128, 2048], F32R, tag="mw2t", name="mw2t", bufs=2)
                    nc.sync.dma_start(wt, dap(mw2_d[b], s * 2048,
                                              [[9 * 2048, 128], [1, 2048]]))
                    for cit in range(8):
                        for ct in range(2):
                            for nh in range(2):
                                nc.tensor.matmul(
                                    pss[(ct, nh)],
                                    wt[:, cit * 256 + ct * 128:cit * 256 + (ct + 1) * 128],
                                    conv_shift_ap(hm[cit], 30, ky, kx, 1, nh),
                                    start=(s == 0 and cit == 0),
                                    stop=(s == 8 and cit == 7))
                for ct in range(2):
                    for nh in range(2):
                        sl = slice(nh * 392, (nh + 1) * 392)
                        nc.vector.scalar_tensor_tensor(
                            res[ct][:, sl], pss[(ct, nh)],
                            bias_ap(34 + b * 2 + ct),
                            res[ct][:, sl], op0=OP.add, op1=OP.add)

                dump_tap(f"res_b{b}")

            # ================= FINAL LAYERNORM =================
            for ct in range(2):
                stats = sb.tile([128, 2, 6], F32, tag="lnstats", name="lnstats")
                for i in range(2):
                    nc.vector.bn_stats(stats[:, i, :], res[ct][:, i * 392:(i + 1) * 392])
                mv = sb.tile([128, 2], F32, tag="lnmv", name="lnmv")
                nc.vector.bn_aggr(mv, stats)
                sd = sb.tile([128, 1], F32, tag="lnsd", name="lnsd")
                nc.scalar.activation(sd, mv[:, 1:2], AF.Sqrt, bias=epst, scale=1.0)
                rs = sb.tile([128, 1], F32, tag="lnrs", name="lnrs")
                nc.vector.reciprocal(rs, sd)
                t1 = sb.tile([128, 784], F32, tag="sd_b", name="lnt1", bufs=1)
                nc.vector.tensor_scalar(t1, res[ct], mv[:, 0:1], rs,
                                        OP.subtract, OP.mult)
                t2 = sb.tile([128, 784], F32, tag="tmp", name="lnt2")
                nc.vector.tensor_mul(t2, t1, lnfw_b)
                t3 = sb.tile([128, 784], F32, tag="m_b", name="lnt3", bufs=1)
                nc.vector.tensor_add(t3, t2, lnfb_b)
                nc.sync.dma_start(out_d[ct * 128:(ct + 1) * 128, :], t3)

    nc.compile()
    return nc


_CACHE = {}


def _get_nc():
    if "nc" not in _CACHE:
        _CACHE["nc"] = build()
    return _CACHE["nc"]


def _run(x, params, trace=False):
    from concourse import bass_utils
    xs, shared = _prep(x, params)
    nc = _get_nc()
    in_maps = [dict(shared, x=xs[i]) for i in range(8)]
    res = bass_utils.run_bass_kernel_spmd(nc, in_maps, core_ids=list(range(8)),
                                          trace=trace)
    out = np.stack([res.results[c]["out"].reshape(256, 28, 28) for c in range(8)])
    return out.astype(np.float32), res


def kernel(x, params):
    out, _ = _run(x, params, trace=False)
    return out
